# revision 71
# baseline (speedup 1.0000x reference)
"""KDE HyperGraph Conv kernel for 8 Trainium2 NeuronCores.

Math: the reference builds H[i,j] = [rho_i > rho_j] (+ self loop), so every
downstream quantity depends only on the *ranks* of the KDE densities rho.
With r_i = rank(rho_i) (ascending) and g_i = M-1-r_i (# of larger rhos):

    De_j = g_j + 1,  Dv_i = M - g_i
    A[i,k] = C(min(r_i, r_k)),  C(r) = H_M - H_{M-1-r}   (harmonic numbers)
    y_i = dvs_i * (  sum_{r_k < r_i} C_k dvs_k x_k
                   + C_i * sum_{r_k >= r_i} dvs_k x_k ),  dvs = Dv^-1/2

which turns the [M,M]@[M,M] propagation into two [M,M]@[M,C] matmuls with
the 0/1 comparison matrix L[i,k] = [rho_k < rho_i] plus elementwise work.
C(r) is evaluated with the asymptotic expansion of harmonic numbers.

Perf notes vs the previous version:
  * All heavy matmuls (Gram, mask col-sums, L@u / L@v) run in bf16 —
    fp32r MACs draw ~4x the power and tripped the PE's power throttle
    (HAM k=4/8 half-clock for ~48% of the kernel). The 0/1 masks are
    exact in bf16; bf16 Gram noise on rho (~3e-6 rel) sits below the
    adjacent-rank gaps (~6e-5), so ranks are essentially unaffected.
  * The comparison tiles L are generated ONCE per batch (bf16, 16-slot
    SBUF ring) and consumed by both the rank col-sums and the L@u/L@v
    pass; generation is split across the Vector and GpSimd engines.
  * Broadcast-copy work moved off the Scalar engine onto GpSimd.
  * h is estimated from a 2-chunk (256-row) distance sample (rel err
    ~6e-5, far below what would perturb any rank).

Data-parallel over batch: each of the 8 cores handles B/8 = 2 batches.
"""

import os
import sys

for _p in ("/opt/trn_rl_repo",):
    if os.path.isdir(_p) and _p not in sys.path:
        sys.path.append(_p)

import numpy as np

import concourse.bass as bass
import concourse.tile as tile
from concourse import bacc, mybir
from concourse.bass_utils import run_bass_kernel_spmd
from concourse.masks import make_identity

N_CORES = 8
B, M, C = 16, 2048, 128
NB = B // N_CORES          # batches per core
NT = M // 128              # 128-row chunks per batch
NS = M // 512              # 512-wide column slices
EPS = 0.2                  # diagonal-safety shift added inside sqrt; sq is
                           # computed from the same bf16 x as the Gram, so
                           # the diag residual is only f32r rounding (<0.1)
GAMMA = 0.5772156649015329
H_M = float((1.0 / np.arange(1, M + 1, dtype=np.float64)).sum())
HS = (0, 8)                # chunks sampled for the bandwidth h

DEBUG_TAPS = False

F32 = mybir.dt.float32
BF16 = mybir.dt.bfloat16
F8 = mybir.dt.float8e4
F32R = mybir.dt.float32r
AF = mybir.ActivationFunctionType
ALU = mybir.AluOpType
AX = mybir.AxisListType

DR = mybir.MatmulPerfMode.DoubleRow


def _tb(t):
    return slice(t * 128, (t + 1) * 128)


def _sl(j):
    return slice(j * 512, (j + 1) * 512)


def build_kernel():
    nc = bacc.Bacc("TRN2", target_bir_lowering=False, debug=False)

    # Per-core inputs (host pre-permuted, see make_in_maps below):
    #   xT[b, c, i]          = x[b, i, c]            (channels on partitions)
    #   xN[b, p, t*128 + c]  = x[b, t*128 + p, c]    (rows on partitions, chunked)
    #   WT[c, o]             = W[o, c]               (projection, pre-transposed)
    # Output yH[b, o, i] = y[b, i, o]  (transposed; host un-permutes)
    xT = nc.declare_dram_parameter("xT", [NB, 128, M], F32, isOutput=False)
    xN = nc.declare_dram_parameter("xN", [NB, 128, M], F32, isOutput=False)
    WT = nc.declare_dram_parameter("WT", [128, 128], F32, isOutput=False)
    yH = nc.declare_dram_parameter("yH", [NB, 128, M], F32, isOutput=True)
    if DEBUG_TAPS:
        dbg_rho = nc.declare_dram_parameter(
            "dbg_rho", [NB, 128, NT], F32, isOutput=True
        )
        dbg_racc = nc.declare_dram_parameter(
            "dbg_racc", [NB, 128, NT], F32, isOutput=True
        )
        dbg_h = nc.declare_dram_parameter("dbg_h", [NB, 128, 1], F32, isOutput=True)
        dbg_z = nc.declare_dram_parameter("dbg_z", [NB, 128, M], F32, isOutput=True)
        dbg_hacc = nc.declare_dram_parameter(
            "dbg_hacc", [NB, 128, len(HS)], F32, isOutput=True
        )
        dbg_sqrow = nc.declare_dram_parameter(
            "dbg_sqrow", [NB, 1, M], F32, isOutput=True
        )
        dbg_biass = nc.declare_dram_parameter(
            "dbg_biass", [NB, 128, NT], F32, isOutput=True
        )
        dbg_scr = nc.declare_dram_parameter(
            "dbg_scr", [NB, 128, M], F8, isOutput=True
        )
        dbg_d2 = nc.declare_dram_parameter(
            "dbg_d2", [NB, 128, M], F32, isOutput=True
        )

    with tile.TileContext(nc) as tc:
        from contextlib import ExitStack

        with ExitStack() as ctx:
            cp = ctx.enter_context(tc.tile_pool(name="consts", bufs=1))
            pb = ctx.enter_context(tc.tile_pool(name="big", bufs=2))
            psm = ctx.enter_context(tc.tile_pool(name="small", bufs=2))
            pp = ctx.enter_context(
                tc.tile_pool(name="psum", bufs=2, space=bass.MemorySpace.PSUM)
            )

            ones_col = cp.tile([128, 1], F32, tag="ones_col")
            nc.gpsimd.memset(ones_col[:, :], 1.0)
            ones_row = cp.tile([1, 128], F32, tag="ones_row")
            nc.gpsimd.memset(ones_row[:, :], 1.0)
            ones_row_r = cp.tile([1, 128], F32R, tag="ones_row_r")
            nc.vector.tensor_copy(ones_row_r[:, :], ones_row[:, :])
            mhalf_f32 = cp.tile([1, 128], F32, tag="mhalf_f32")
            nc.gpsimd.memset(mhalf_f32[:, :], -0.5)
            mhalf_row = cp.tile([1, 128], F32R, tag="mhalf_row")
            nc.vector.tensor_copy(mhalf_row[:, :], mhalf_f32[:, :])
            ones_col_r = cp.tile([128, 2], F32R, tag="ones_col_r")
            nc.vector.tensor_copy(ones_col_r[:, 0:1], ones_col[:, :])
            nc.vector.tensor_copy(ones_col_r[:, 1:2], ones_col[:, :])
            ones8 = cp.tile([128, 2, 128], F8, tag="ones8")
            nc.gpsimd.memset(ones8[:, :, :], 1.0)
            ones2_b = cp.tile([128, 2], BF16, tag="ones2_b")
            nc.gpsimd.memset(ones2_b[:, :], 1.0)
            o128_f = cp.tile([128, 128], F32, tag="o128_f")
            nc.gpsimd.memset(o128_f[:, :], 1.0 / 128.0)
            oneon128_r = cp.tile([128, 128], F32R, tag="oneon128_r")
            nc.vector.tensor_copy(oneon128_r[:, :], o128_f[:, :])
            ident = cp.tile([128, 128], F32, tag="ident")
            make_identity(nc, ident[:, :])
            wt_sb = cp.tile([128, 128], F32, tag="wt")
            nc.sync.dma_start(wt_sb[:, :], WT[:, :])
            wt_r = cp.tile([128, 128], F32R, tag="wt_r")
            nc.vector.tensor_copy(wt_r[:, :], wt_sb[:, :])

            # short dense burst of real-shaped matmuls to ramp the PE
            # p-state before the latency-critical phases start
            junk = cp.tile([128, 512], BF16, tag="junk")
            nc.gpsimd.memset(junk[:, :], 0.5)
            warm_ps = pp.tile([128, 512], F32, tag="big", name="warmps")
            for _w in range(12):
                nc.tensor.matmul(
                    warm_ps[:, :], lhsT=junk[:, 0:128], rhs=junk[:, :],
                    start=True, stop=True, skip_group_check=True,
                )

            st = [dict() for _ in range(NB)]
            lts = [[None] * (NT // 2) for _ in range(NB)]

            # ---------- phase: prep (loads, squares, sq row + chunk) ----------
            def emit_prep(b):
                s = st[b]
                xt = pb.tile([128, M], F32, tag="xt", bufs=1, name=f"xt{b}")
                xb = pb.tile([128, M], BF16, tag="xb", name=f"xb{b}")
                xsq = pb.tile([128, M], F32R, tag="xsq", bufs=1, name=f"xsq{b}")
                sq_ps = pp.tile([1, M], F32, tag="big", name=f"sqps{b}")
                # slice-wise load/cast/square pipeline: the first Gram can
                # start as soon as slice 0 has been cast
                for j in range(NS):
                    nc.sync.dma_start(xt[:, _sl(j)], xT[b][:, _sl(j)])
                    nc.vector.tensor_copy(xb[:, _sl(j)], xt[:, _sl(j)])
                    nc.scalar.activation(
                        xsq[:, _sl(j)], xb[:, _sl(j)], AF.Square
                    )
                    nc.tensor.matmul(
                        sq_ps[:, _sl(j)], lhsT=ones_col_r[:, 0:1], rhs=xsq[:, _sl(j)]
                    )
                sq_row = psm.tile([1, M], F32R, tag="sq_row", name=f"sqrow{b}")
                nc.scalar.copy(sq_row[0:1, :], sq_ps[:, :])
                # sq in chunk layout [128, NT] via DMA + PE transpose so the
                # ACT bias sees the same PE-computed sq as the PSUM d2'
                s16 = psm.tile([NT, 128], F32, tag="s16", name=f"s16_{b}")
                nc.sync.dma_start(s16[:, :], sq_row[0:1, :].bitcast(F32))
                sqc_ps = pp.tile([128, NT], F32, tag="big", name=f"sqcps{b}")
                nc.tensor.transpose(sqc_ps[:, :], s16[:, :], ident[0:NT, 0:NT])
                sqc = psm.tile([128, NT], F32, tag="sqc", name=f"sqc{b}")
                nc.scalar.copy(sqc[:, :], sqc_ps[:, :])
                bias_s = psm.tile([128, NT], F32, tag="bias_s", name=f"biass{b}")
                nc.vector.tensor_scalar_add(bias_s[:, :], sqc[:, :], EPS)
                # msq[p, j] = -sq_j/2 replicated across partitions, consumed
                # by a K=128 (1/128-weighted) PSUM-init matmul: a K=1 rank-1
                # init would drop the PE activity monitor to half utilization
                # and halve the clock for the whole distance pass.
                msqrow = psm.tile([1, M], F32R, tag="rowtmp", bufs=1,
                                  name=f"msqr{b}")
                nc.vector.tensor_scalar_mul(msqrow[0:1, :], sq_row[0:1, :], -0.5)
                msq = pb.tile([128, M], F32R, tag="msq", bufs=2, name=f"msq{b}")
                nc.gpsimd.partition_broadcast(msq[:, :], msqrow[0:1, :])
                s["xb"], s["sq_row"], s["sqc"], s["bias_s"] = (
                    xb, sq_row, sqc, bias_s,
                )
                s["msq"] = msq

            def emit_xn(b):
                s = st[b]
                xn = pb.tile([128, M], F32, tag="xn", bufs=1, name=f"xn{b}")
                nc.sync.dma_start(xn[:, :], xN[b])
                s["xn"] = xn

            # ---------- phase: pass B (sampled dist sums for h) --------------
            # h is a mean over 4.2M off-diag distances; a 2-chunk (256-row)
            # sample estimates it to ~6e-5 relative, far below the spacing
            # of adjacent rho values, so ranks are unaffected.
            def emit_passB(b):
                s = st[b]
                hacc = psm.tile([128, len(HS)], F32, tag="hacc", name=f"hacc{b}")
                for hi, t in enumerate(HS):
                    d2_ps = pp.tile([128, M], F32, tag="big", name=f"d2b{b}_{t}")
                    # Gram first (needs only x), -sq/2 init last: hides the
                    # sq-row/msq latency chain behind the Gram matmuls
                    for j in range(NS):
                        nc.tensor.matmul(
                            d2_ps[:, _sl(j)],
                            lhsT=s["xb"][:, _tb(t)],
                            rhs=s["xb"][:, _sl(j)],
                            start=True,
                            stop=False,
                        )
                    for j in range(NS):
                        nc.tensor.matmul(
                            d2_ps[:, _sl(j)],
                            lhsT=oneon128_r[:, :],
                            rhs=s["msq"][:, _sl(j)],
                            start=False,
                            stop=True,
                        )
                    scr = pb.tile([128, M], F8, tag="scr", bufs=1, name=f"sb{b}_{t}")
                    nc.scalar.activation(
                        scr[:, :],
                        d2_ps[:, :],
                        AF.Sqrt,
                        scale=-2.0,
                        bias=s["bias_s"][:, t : t + 1],
                        accum_out=hacc[:, hi : hi + 1],
                    )
                    if DEBUG_TAPS and t == HS[0]:
                        nc.sync.dma_start(dbg_scr[b], scr[:, :])
                s["hacc"] = hacc
                if DEBUG_TAPS:
                    nc.sync.dma_start(dbg_hacc[b], hacc[:, :])
                    nc.sync.dma_start(dbg_sqrow[b], s["sq_row"][0:1, :].bitcast(F32))
                    nc.sync.dma_start(dbg_biass[b], s["bias_s"][:, :])

            # ---------- phase: h -> f2 = 1/h^2, exp bias -------------------
            def emit_h(b):
                s = st[b]
                hsum = psm.tile([128, 1], F32, tag="hsum", name=f"hsum{b}")
                nc.vector.tensor_reduce(
                    hsum[:, :], s["hacc"][:, :], axis=AX.X, op=ALU.add
                )
                tot_ps = pp.tile([1, 1], F32, tag="big", name=f"tot{b}")
                nc.tensor.matmul(tot_ps[:, :], lhsT=ones_col[:, :], rhs=hsum[:, :])
                ht = psm.tile([1, 1], F32, tag="ht", name=f"ht{b}")
                nc.vector.tensor_scalar(
                    ht[:, :],
                    tot_ps[:, :],
                    1.0 / (len(HS) * 128 * (M - 1)),
                    1e-6,
                    op0=ALU.mult,
                    op1=ALU.max,
                )
                # remove the EPS-induced bias: E[sqrt(d2+EPS)] ~ sqrt(d2)
                # + EPS/(2 h), so h_corr^2 = h^2 - EPS to second order.
                # All-DVE chain (no cross-engine hops until the broadcast).
                h2m = psm.tile([1, 1], F32, tag="h2m", name=f"h2m{b}")
                nc.vector.tensor_mul(h2m[:, :], ht[:, :], ht[:, :])
                h2e = psm.tile([1, 1], F32, tag="h2e", name=f"h2e{b}")
                nc.vector.tensor_scalar_add(h2e[:, :], h2m[:, :], -EPS)
                rh2 = psm.tile([1, 1], F32, tag="rh2", name=f"rh2{b}")
                nc.vector.reciprocal(rh2[:, :], h2e[:, :])
                f_bc = psm.tile([128, 1], F32, tag="fbc", name=f"fbc{b}")
                nc.gpsimd.partition_broadcast(f_bc[:, :], rh2[0:1, :])
                if DEBUG_TAPS:
                    nc.sync.dma_start(dbg_h[b], f_bc[:, :])
                # exp arg = f2*d2' - f2*sq_i/2  (the dropped EPS is a global
                # factor on every rho -> cannot change any comparison)
                bias_e = psm.tile([128, NT], F32, tag="bias_e", name=f"biase{b}")
                nc.vector.tensor_scalar(
                    bias_e[:, :], s["sqc"][:, :], f_bc[:, 0:1], -0.5,
                    op0=ALU.mult, op1=ALU.mult,
                )
                s["f_bc"], s["bias_e"] = f_bc, bias_e

            # ---------- phase: pass C (rho row-sums via exp accumulate) ------
            # PSUM is initialized with the -sq_j/2 row via a K=1 f32r rank-1
            # matmul (exact enough: per-column bias errors would directly
            # perturb rho ranks, so this path stays off bf16), then the
            # bf16 Gram accumulates on top.
            def emit_passC_pre(b):
                s = st[b]
                rho = psm.tile([128, NT], F32, tag="rho", name=f"rho{b}")
                s["rho"] = rho

            def emit_passC_chunk(b, t):
                s = st[b]
                d2_ps = pp.tile([128, M], F32, tag="big", name=f"d2c{b}_{t}")
                for j in range(NS):
                    nc.tensor.matmul(
                        d2_ps[:, _sl(j)], lhsT=s["xb"][:, _tb(t)],
                        rhs=s["xb"][:, _sl(j)], start=True, stop=False,
                    )
                for j in range(NS):
                    nc.tensor.matmul(
                        d2_ps[:, _sl(j)], lhsT=oneon128_r[:, :],
                        rhs=s["msq"][:, _sl(j)], start=False, stop=True,
                    )
                scr = pb.tile([128, M], F8, tag="scr", bufs=1, name=f"sc{b}_{t}")
                nc.scalar.activation(
                    scr[:, :],
                    d2_ps[:, :],
                    AF.Exp,
                    scale=s["f_bc"][:, :],
                    bias=s["bias_e"][:, t : t + 1],
                    accum_out=s["rho"][:, t : t + 1],
                )

            # ---------- per batch building blocks ----------
            def emit_layout(b):
                """rho as an exact [1, M] row and [128, M] broadcast tile."""
                s = st[b]
                rT_ps = pp.tile([NT, 128], F32, tag="big", name=f"rTps{b}")
                nc.tensor.transpose(rT_ps[:, :], s["rho"][:, :], ident[:, :])
                rT = psm.tile([NT, 128], F32, tag="rT", name=f"rT{b}")
                nc.scalar.copy(rT[:, :], rT_ps[:, :])
                rrow = psm.tile([1, M], F32, tag="rowtmp", bufs=1, name=f"rrow{b}")
                nc.sync.dma_start(rrow[0:1, :], rT[:, :])
                rho_bc = pb.tile([128, M], F32, tag="rho_bc", bufs=2, name=f"rbc{b}")
                nc.gpsimd.partition_broadcast(rho_bc[:, :], rrow[0:1, :])
                s["rho_bc"] = rho_bc

            def alloc_lt(b):
                for k in range(NT // 2):
                    lts[b][k] = pb.tile([128, 2, M], F8, tag="lt", bufs=NT,
                                        name=f"lt{b}_{k}")

            def emit_ltgen(b, t):
                """one fp8 comparison tile L[p, j] = [rho_j > rho_{t*128+p}],
                written into half t%2 of the chunk-pair tile t//2."""
                s = st[b]
                nc.vector.tensor_scalar(
                    lts[b][t // 2][:, t % 2, :], s["rho_bc"][:, :],
                    s["rho"][:, t : t + 1], None, op0=ALU.is_gt,
                )

            def emit_colsum_mm(b):
                """ranks r_i = #{k: rho_k < rho_i}: column-sum the stored
                compare tiles on the PE (fp8 DoubleRow: one chunk pair per
                pass)."""
                s = st[b]
                r_ps = pp.tile([128, M], F32, tag="big", name=f"rps{b}")
                for k in range(NT // 2):
                    for j in range(NS):
                        nc.tensor.matmul(
                            r_ps[:, _sl(j)], lhsT=ones8[:, :, :],
                            rhs=lts[b][k][:, :, _sl(j)],
                            start=(k == 0), stop=(k == NT // 2 - 1),
                            perf_mode=DR,
                        )
                s["r_ps"] = r_ps

            def emit_colsum_post(b):
                """move the rank row to chunk layout [128, NT]."""
                s = st[b]
                r_sb = psm.tile([1, M], F32, tag="rowtmp", bufs=1, name=f"rsb{b}")
                nc.vector.tensor_copy(r_sb[0:1, :], s["r_ps"][0:1, :])
                r16 = psm.tile([NT, 128], F32, tag="r16", name=f"r16_{b}")
                nc.sync.dma_start(r16[:, :], r_sb[0:1, :])
                rc_ps = pp.tile([128, NT], F32, tag="big", name=f"rcps{b}")
                nc.tensor.transpose(rc_ps[:, :], r16[:, :], ident[0:NT, 0:NT])
                racc = psm.tile([128, NT], F32, tag="racc", name=f"racc{b}")
                nc.scalar.copy(racc[:, :], rc_ps[:, :])
                s["racc"] = racc
                if DEBUG_TAPS:
                    nc.sync.dma_start(dbg_racc[b], racc[:, :])

            def emit_rank_chain(b):
                """Dv/dvs/C vectors from ranks."""
                s = st[b]
                racc = s["racc"]
                # engine-grouped to cut cross-engine dependency hops:
                # DVE batch -> ACT batch -> DVE batch
                Dv = psm.tile([128, NT], F32, tag="Dv", name=f"Dv{b}")
                nc.vector.tensor_scalar_add(Dv[:, :], racc[:, :], 1.0)
                g = psm.tile([128, NT], F32, tag="g", name=f"g{b}")
                nc.vector.tensor_scalar(
                    g[:, :], racc[:, :], -1.0, float(M - 1), op0=ALU.mult,
                    op1=ALU.add,
                )
                gm = psm.tile([128, NT], F32, tag="gm", name=f"gm{b}")
                nc.vector.tensor_scalar_max(gm[:, :], g[:, :], 1.0)
                inv = psm.tile([128, NT], F32, tag="inv", name=f"inv{b}")
                nc.vector.reciprocal(inv[:, :], gm[:, :])
                lnDv = psm.tile([128, NT], F32, tag="lnDv", name=f"lnDv{b}")
                nc.scalar.activation(lnDv[:, :], Dv[:, :], AF.Ln)
                lng = psm.tile([128, NT], F32, tag="lng", name=f"lng{b}")
                nc.scalar.activation(lng[:, :], gm[:, :], AF.Ln)
                dvs = psm.tile([128, NT], F32, tag="dvs", name=f"dvs{b}")
                nc.scalar.activation(dvs[:, :], lnDv[:, :], AF.Exp, scale=-0.5)
                inv2 = psm.tile([128, NT], F32, tag="inv2", name=f"inv2{b}")
                nc.vector.tensor_mul(inv2[:, :], inv[:, :], inv[:, :])
                c1 = psm.tile([128, NT], F32, tag="c1", name=f"c1{b}")
                nc.vector.tensor_scalar(
                    c1[:, :], lng[:, :], -1.0, H_M - GAMMA, op0=ALU.mult, op1=ALU.add
                )
                c2 = psm.tile([128, NT], F32, tag="c2", name=f"c2{b}")
                nc.vector.scalar_tensor_tensor(
                    c2[:, :], in0=inv[:, :], scalar=-0.5, in1=c1[:, :],
                    op0=ALU.mult, op1=ALU.add,
                )
                Cv = psm.tile([128, NT], F32, tag="Cv", name=f"Cv{b}")
                nc.vector.scalar_tensor_tensor(
                    Cv[:, :], in0=inv2[:, :], scalar=1.0 / 12.0, in1=c2[:, :],
                    op0=ALU.mult, op1=ALU.add,
                )
                dvsC = psm.tile([128, NT], F32, tag="dvsC", name=f"dvsC{b}")
                nc.vector.tensor_mul(dvsC[:, :], dvs[:, :], Cv[:, :])
                s["dvs"], s["dvsC"] = dvs, dvsC

            def emit_uvT(b):
                """u, v scaled copies; T column-sums; dvs/dvsC broadcasts."""
                s = st[b]
                dvs, dvsC = s["dvs"], s["dvsC"]
                u = pb.tile([128, M], BF16, tag="u", bufs=1, name=f"u{b}")
                v = pb.tile([128, M], BF16, tag="v", bufs=1, name=f"v{b}")
                for t in range(NT):
                    nc.vector.tensor_scalar(
                        u[:, _tb(t)], s["xn"][:, _tb(t)], dvs[:, t : t + 1], None,
                        op0=ALU.mult,
                    )
                    nc.vector.tensor_scalar(
                        v[:, _tb(t)], s["xn"][:, _tb(t)], dvsC[:, t : t + 1], None,
                        op0=ALU.mult,
                    )
                s["u"], s["v"] = u, v
                T_ps = pp.tile([128, 2], F32, tag="big", name=f"Tps{b}")
                for t in range(NT):
                    nc.tensor.matmul(
                        T_ps[:, :], lhsT=u[:, _tb(t)], rhs=ones2_b[:, :],
                        start=(t == 0), stop=(t == NT - 1),
                    )
                T_sb = psm.tile([128, 1], F32, tag="T_sb", name=f"Tsb{b}")
                nc.scalar.copy(T_sb[:, :], T_ps[:, 0:1])
                s["T_sb"] = T_sb

                stk = psm.tile([128, 2 * NT], F32, tag="stk", name=f"stk{b}")
                nc.vector.tensor_copy(stk[:, 0:NT], dvs[:, :])
                # negated: the projection accumulates W@(dvs*P1) + W@(-dvsC*(P2-T))
                nc.vector.tensor_scalar_mul(stk[:, NT : 2 * NT], dvsC[:, :], -1.0)
                stT_ps = pp.tile([2 * NT, 128], F32, tag="big", name=f"stTps{b}")
                nc.tensor.transpose(stT_ps[:, :], stk[:, :], ident[:, :])
                stT = psm.tile([2 * NT, 128], F32R, tag="stT", name=f"stT{b}")
                nc.vector.tensor_copy(stT[:, :], stT_ps[:, :])
                dvs_row = psm.tile([1, M], F32R, tag="rowtmp", bufs=1,
                                   name=f"dr{b}")
                nc.sync.dma_start(dvs_row[0:1, :], stT[0:NT, :])
                dvsC_row = psm.tile([1, M], F32R, tag="rowtmp", bufs=1,
                                    name=f"cr{b}")
                nc.sync.dma_start(dvsC_row[0:1, :], stT[NT : 2 * NT, :])

                dvs_bc = pb.tile([128, M], F32, tag="dvs_bc", bufs=1, name=f"db{b}")
                nc.gpsimd.partition_broadcast(
                    dvs_bc[:, :], dvs_row[0:1, :].bitcast(F32)
                )
                dvsC_bc = pb.tile([128, M], F32, tag="dvsC_bc", bufs=1,
                                  name=f"cb{b}")
                nc.gpsimd.partition_broadcast(
                    dvsC_bc[:, :], dvsC_row[0:1, :].bitcast(F32)
                )
                s["dvs_bc"], s["dvsC_bc"] = dvs_bc, dvsC_bc

            def emit_l2_half(b, jh, P12_ps, ltgen_tail=None):
                """one j-half of the propagation: P12 packs [P2h | P1h];
                P2h[:, j] = sum_i u_i lt[i,j], P1h likewise with v."""
                s = st[b]
                for t in range(NT):
                    ltp, h = lts[b][t // 2], t % 2
                    for js in range(2):
                        sl = slice(js * 512, (js + 1) * 512)
                        lsl = slice(jh * 1024 + js * 512, jh * 1024 + (js + 1) * 512)
                        nc.tensor.matmul(
                            P12_ps[:, sl], lhsT=s["u"][:, _tb(t)],
                            rhs=ltp[:, h, lsl],
                            start=(t == 0), stop=(t == NT - 1),
                        )
                    for js in range(2):
                        sl = slice(1024 + js * 512, 1024 + (js + 1) * 512)
                        lsl = slice(jh * 1024 + js * 512, jh * 1024 + (js + 1) * 512)
                        nc.tensor.matmul(
                            P12_ps[:, sl], lhsT=s["v"][:, _tb(t)],
                            rhs=ltp[:, h, lsl],
                            start=(t == 0), stop=(t == NT - 1),
                        )
                    if ltgen_tail is not None:
                        ltgen_tail(t)

            def emit_z_half(b, jh, P12_ps):
                """zt1_h = -(dvs*C)_i*(P2h - T_c), zt2_h = dvs_i*P1h."""
                s = st[b]
                hsl = slice(jh * 1024, (jh + 1) * 1024)
                zt2 = pb.tile([128, 1024], F32R, tag="u", bufs=1,
                              name=f"zt2{b}_{jh}")
                nc.vector.scalar_tensor_tensor(
                    zt2[:, :], in0=P12_ps[:, 1024:2048], scalar=0.0,
                    in1=s["dvs_bc"][:, hsl], op0=ALU.bypass, op1=ALU.mult,
                )
                zt1 = pb.tile([128, 1024], F32R, tag="v", bufs=1,
                              name=f"zt1{b}_{jh}")
                nc.vector.scalar_tensor_tensor(
                    zt1[:, :], in0=P12_ps[:, 0:1024], scalar=s["T_sb"][:, 0:1],
                    in1=s["dvsC_bc"][:, hsl], op0=ALU.subtract, op1=ALU.mult,
                )
                s[f"zt{jh}"] = (zt1, zt2)
                if DEBUG_TAPS:
                    zdbg = pb.tile([128, 1024], F32, tag="zdbg", bufs=1,
                                   name=f"zdbg{b}_{jh}")
                    nc.vector.tensor_add(zdbg[:, :], zt2[:, :].bitcast(F32),
                                         zt1[:, :].bitcast(F32))
                    nc.sync.dma_start(dbg_z[b][:, hsl], zdbg[:, :])

            def emit_proj_half(b, jh):
                """yT_h = W @ (zt2_h + zt1_h) via PSUM accumulation; SiLU."""
                s = st[b]
                zt1, zt2 = s[f"zt{jh}"]
                yT_ps = pp.tile([128, 1024], F32, tag="big", name=f"yTps{b}_{jh}")
                for js in range(2):
                    sl = slice(js * 512, (js + 1) * 512)
                    nc.tensor.matmul(
                        yT_ps[:, sl], lhsT=wt_r[:, :], rhs=zt2[:, sl],
                        start=True, stop=False,
                    )
                    nc.tensor.matmul(
                        yT_ps[:, sl], lhsT=wt_r[:, :], rhs=zt1[:, sl],
                        start=False, stop=True,
                    )
                y_sb = pb.tile([128, 1024], F32, tag="y_sb", bufs=2,
                               name=f"ysb{b}_{jh}")
                nc.scalar.activation(y_sb[:, :], yT_ps[:, :], AF.Silu)
                nc.sync.dma_start(
                    yH[b][:, jh * 1024 : (jh + 1) * 1024], y_sb[:, :]
                )

            # ---------- schedule ----------
            # batch-0 mask generation (DVE) rides under batch-1's distance
            # pass (PE/ACT); batch-1 mask generation rides under batch-0's
            # L@u/L@v pass. The propagation/projection runs in j-halves so
            # the z/proj chain of one half overlaps PE work of the next.
            emit_prep(0)
            emit_prep(1)
            # filler matmuls: keep the PE activity monitor boosted while the
            # prep DMA/cast/square chain runs (idle PE halves the clock)
            warm2_ps = pp.tile([128, 512], F32, tag="big", name="warmps2")
            for _w in range(20):
                nc.tensor.matmul(
                    warm2_ps[:, :], lhsT=junk[:, 0:128], rhs=junk[:, :],
                    start=True, stop=True, skip_group_check=True,
                )
            emit_passB(0)
            emit_passB(1)
            emit_h(0)
            emit_xn(0)
            emit_xn(1)
            emit_passC_pre(0)
            for t in range(4):
                emit_passC_chunk(0, t)
            emit_h(1)
            for t in range(4, NT):
                emit_passC_chunk(0, t)
            if DEBUG_TAPS:
                nc.sync.dma_start(dbg_rho[0], st[0]["rho"][:, :])
            emit_layout(0)
            emit_passC_pre(1)
            alloc_lt(0)
            for t in range(NT):
                emit_passC_chunk(1, t)
                emit_ltgen(0, t)
            if DEBUG_TAPS:
                nc.sync.dma_start(dbg_rho[1], st[1]["rho"][:, :])
            emit_layout(1)
            emit_colsum_mm(0)
            emit_colsum_post(0)
            emit_rank_chain(0)
            emit_uvT(0)
            # batch-1 mask tiles: ring slots 8-15 are free (full double
            # buffer), so generation starts as soon as DVE drains the
            # batch-0 scalar chain; colsum(1) then fills the PE between
            # batch-0's two propagation halves, and batch-1's rank chain
            # runs under batch-0's second half.
            alloc_lt(1)
            for t in range(12):
                emit_ltgen(1, t)
            P12a0 = pp.tile([128, M], F32, tag="big", name="P12a0")
            emit_l2_half(0, 0, P12a0)
            emit_z_half(0, 0, P12a0)
            for t in range(12, NT):
                emit_ltgen(1, t)
            emit_colsum_mm(1)
            P12b0 = pp.tile([128, M], F32, tag="big", name="P12b0")
            emit_l2_half(0, 1, P12b0)
            emit_colsum_post(1)
            emit_rank_chain(1)
            emit_z_half(0, 1, P12b0)
            emit_proj_half(0, 0)
            emit_proj_half(0, 1)
            emit_uvT(1)
            P12a1 = pp.tile([128, M], F32, tag="big", name="P12a1")
            emit_l2_half(1, 0, P12a1)
            emit_z_half(1, 0, P12a1)
            P12b1 = pp.tile([128, M], F32, tag="big", name="P12b1")
            emit_l2_half(1, 1, P12b1)
            emit_proj_half(1, 0)
            emit_z_half(1, 1, P12b1)
            emit_proj_half(1, 1)

    nc.compile()
    return nc


_CACHED_NC = None


def _get_nc():
    global _CACHED_NC
    if _CACHED_NC is None:
        _CACHED_NC = build_kernel()
    return _CACHED_NC


def make_in_maps(x, W):
    x = np.asarray(x, dtype=np.float32)
    W = np.asarray(W, dtype=np.float32)
    wt = np.ascontiguousarray(W.T)
    in_maps = []
    for core in range(N_CORES):
        xb = x[core * NB : (core + 1) * NB]                       # [NB, M, C]
        xt = np.ascontiguousarray(xb.transpose(0, 2, 1))          # [NB, C, M]
        # xn[b, p, t*128+c] = x[b, t*128+p, c]
        xn = np.ascontiguousarray(
            xb.reshape(NB, NT, 128, C).transpose(0, 2, 1, 3).reshape(NB, 128, M)
        )
        in_maps.append({"xT": xt, "xN": xn, "WT": wt})
    return in_maps


def unshard_output(results):
    outs = []
    for core in range(N_CORES):
        yh = results[core]["yH"]                                  # [NB, C, M]
        outs.append(yh.transpose(0, 2, 1))                        # [NB, M, C]
    return np.concatenate(outs, axis=0).astype(np.float32)


def run(x, W, trace=False, trace_kwargs=None):
    nc = _get_nc()
    res = run_bass_kernel_spmd(
        nc,
        make_in_maps(x, W),
        list(range(N_CORES)),
        trace=trace,
        **(trace_kwargs or {}),
    )
    return unshard_output(res.results), res


def kernel(x, W):
    y, _ = run(x, W, trace=False)
    return y


# revision 74
# speedup vs baseline: 1.0147x; 1.0147x over previous
"""KDE HyperGraph Conv kernel for 8 Trainium2 NeuronCores.

Math: the reference builds H[i,j] = [rho_i > rho_j] (+ self loop), so every
downstream quantity depends only on the *ranks* of the KDE densities rho.
With r_i = rank(rho_i) (ascending) and g_i = M-1-r_i (# of larger rhos):

    De_j = g_j + 1,  Dv_i = M - g_i
    A[i,k] = C(min(r_i, r_k)),  C(r) = H_M - H_{M-1-r}   (harmonic numbers)
    y_i = dvs_i * (  sum_{r_k < r_i} C_k dvs_k x_k
                   + C_i * sum_{r_k >= r_i} dvs_k x_k ),  dvs = Dv^-1/2

which turns the [M,M]@[M,M] propagation into two [M,M]@[M,C] matmuls with
the 0/1 comparison matrix L[i,k] = [rho_k < rho_i] plus elementwise work.
C(r) is evaluated with the asymptotic expansion of harmonic numbers.

Perf notes vs the previous version:
  * All heavy matmuls (Gram, mask col-sums, L@u / L@v) run in bf16 —
    fp32r MACs draw ~4x the power and tripped the PE's power throttle
    (HAM k=4/8 half-clock for ~48% of the kernel). The 0/1 masks are
    exact in bf16; bf16 Gram noise on rho (~3e-6 rel) sits below the
    adjacent-rank gaps (~6e-5), so ranks are essentially unaffected.
  * The comparison tiles L are generated ONCE per batch (bf16, 16-slot
    SBUF ring) and consumed by both the rank col-sums and the L@u/L@v
    pass; generation is split across the Vector and GpSimd engines.
  * Broadcast-copy work moved off the Scalar engine onto GpSimd.
  * h is estimated from a 2-chunk (256-row) distance sample (rel err
    ~6e-5, far below what would perturb any rank).

Data-parallel over batch: each of the 8 cores handles B/8 = 2 batches.
"""

import os
import sys

for _p in ("/opt/trn_rl_repo",):
    if os.path.isdir(_p) and _p not in sys.path:
        sys.path.append(_p)

import numpy as np

import concourse.bass as bass
import concourse.tile as tile
from concourse import bacc, mybir
from concourse.bass_utils import run_bass_kernel_spmd
from concourse.masks import make_identity

N_CORES = 8
B, M, C = 16, 2048, 128
NB = B // N_CORES          # batches per core
NT = M // 128              # 128-row chunks per batch
NS = M // 512              # 512-wide column slices
EPS = 0.2                  # diagonal-safety shift added inside sqrt; sq is
                           # computed from the same bf16 x as the Gram, so
                           # the diag residual is only f32r rounding (<0.1)
GAMMA = 0.5772156649015329
H_M = float((1.0 / np.arange(1, M + 1, dtype=np.float64)).sum())
HS = (0, 8)                # chunks sampled for the bandwidth h

DEBUG_TAPS = False

F32 = mybir.dt.float32
BF16 = mybir.dt.bfloat16
F8 = mybir.dt.float8e4
F32R = mybir.dt.float32r
AF = mybir.ActivationFunctionType
ALU = mybir.AluOpType
AX = mybir.AxisListType

DR = mybir.MatmulPerfMode.DoubleRow


def _tb(t):
    return slice(t * 128, (t + 1) * 128)


def _sl(j):
    return slice(j * 512, (j + 1) * 512)


def build_kernel():
    nc = bacc.Bacc("TRN2", target_bir_lowering=False, debug=False)

    # Per-core inputs (host pre-permuted, see make_in_maps below):
    #   xT[b, c, i]          = x[b, i, c]            (channels on partitions)
    #   xN[b, p, t*128 + c]  = x[b, t*128 + p, c]    (rows on partitions, chunked)
    #   WT[c, o]             = W[o, c]               (projection, pre-transposed)
    # Output yH[b, o, i] = y[b, i, o]  (transposed; host un-permutes)
    xT = nc.declare_dram_parameter("xT", [NB, 128, M], F32, isOutput=False)
    xN = nc.declare_dram_parameter("xN", [NB, 128, M], F32, isOutput=False)
    WT = nc.declare_dram_parameter("WT", [128, 128], F32, isOutput=False)
    yH = nc.declare_dram_parameter("yH", [NB, 128, M], F32, isOutput=True)
    if DEBUG_TAPS:
        dbg_rho = nc.declare_dram_parameter(
            "dbg_rho", [NB, 128, NT], F32, isOutput=True
        )
        dbg_racc = nc.declare_dram_parameter(
            "dbg_racc", [NB, 128, NT], F32, isOutput=True
        )
        dbg_h = nc.declare_dram_parameter("dbg_h", [NB, 128, 1], F32, isOutput=True)
        dbg_z = nc.declare_dram_parameter("dbg_z", [NB, 128, M], F32, isOutput=True)
        dbg_hacc = nc.declare_dram_parameter(
            "dbg_hacc", [NB, 128, len(HS)], F32, isOutput=True
        )
        dbg_sqrow = nc.declare_dram_parameter(
            "dbg_sqrow", [NB, 1, M], F32, isOutput=True
        )
        dbg_biass = nc.declare_dram_parameter(
            "dbg_biass", [NB, 128, NT], F32, isOutput=True
        )
        dbg_scr = nc.declare_dram_parameter(
            "dbg_scr", [NB, 128, M], F8, isOutput=True
        )
        dbg_d2 = nc.declare_dram_parameter(
            "dbg_d2", [NB, 128, M], F32, isOutput=True
        )

    with tile.TileContext(nc) as tc:
        from contextlib import ExitStack

        with ExitStack() as ctx:
            cp = ctx.enter_context(tc.tile_pool(name="consts", bufs=1))
            pb = ctx.enter_context(tc.tile_pool(name="big", bufs=2))
            psm = ctx.enter_context(tc.tile_pool(name="small", bufs=2))
            pp = ctx.enter_context(
                tc.tile_pool(name="psum", bufs=2, space=bass.MemorySpace.PSUM)
            )

            ones_col = cp.tile([128, 1], F32, tag="ones_col")
            nc.gpsimd.memset(ones_col[:, :], 1.0)
            ones_row = cp.tile([1, 128], F32, tag="ones_row")
            nc.gpsimd.memset(ones_row[:, :], 1.0)
            ones_row_r = cp.tile([1, 128], F32R, tag="ones_row_r")
            nc.vector.tensor_copy(ones_row_r[:, :], ones_row[:, :])
            mhalf_f32 = cp.tile([1, 128], F32, tag="mhalf_f32")
            nc.gpsimd.memset(mhalf_f32[:, :], -0.5)
            mhalf_row = cp.tile([1, 128], F32R, tag="mhalf_row")
            nc.vector.tensor_copy(mhalf_row[:, :], mhalf_f32[:, :])
            ones_col_r = cp.tile([128, 2], F32R, tag="ones_col_r")
            nc.vector.tensor_copy(ones_col_r[:, 0:1], ones_col[:, :])
            nc.vector.tensor_copy(ones_col_r[:, 1:2], ones_col[:, :])
            ones8 = cp.tile([128, 2, 128], F8, tag="ones8")
            nc.gpsimd.memset(ones8[:, :, :], 1.0)
            ones2_b = cp.tile([128, 2], BF16, tag="ones2_b")
            nc.gpsimd.memset(ones2_b[:, :], 1.0)
            o128_f = cp.tile([128, 128], F32, tag="o128_f")
            nc.gpsimd.memset(o128_f[:, :], 1.0 / 128.0)
            oneon128_r = cp.tile([128, 128], F32R, tag="oneon128_r")
            nc.vector.tensor_copy(oneon128_r[:, :], o128_f[:, :])
            ident = cp.tile([128, 128], F32, tag="ident")
            make_identity(nc, ident[:, :])
            wt_sb = cp.tile([128, 128], F32, tag="wt")
            nc.sync.dma_start(wt_sb[:, :], WT[:, :])
            wt_r = cp.tile([128, 128], F32R, tag="wt_r")
            nc.vector.tensor_copy(wt_r[:, :], wt_sb[:, :])

            # short dense burst of real-shaped matmuls to ramp the PE
            # p-state before the latency-critical phases start
            junk = cp.tile([128, 512], BF16, tag="junk")
            nc.gpsimd.memset(junk[:, :], 0.5)
            warm_ps = pp.tile([128, 512], F32, tag="big", name="warmps")
            for _w in range(12):
                nc.tensor.matmul(
                    warm_ps[:, :], lhsT=junk[:, 0:128], rhs=junk[:, :],
                    start=True, stop=True, skip_group_check=True,
                )

            st = [dict() for _ in range(NB)]
            lts = [[None] * (NT // 2) for _ in range(NB)]

            # ---------- phase: prep (loads, squares, sq row + chunk) ----------
            def emit_prep(b):
                s = st[b]
                xt = pb.tile([128, M], F32, tag="xt", bufs=1, name=f"xt{b}")
                xb = pb.tile([128, M], BF16, tag="xb", name=f"xb{b}")
                xsq = pb.tile([128, M], F32R, tag="xsq", bufs=1, name=f"xsq{b}")
                sq_ps = pp.tile([1, M], F32, tag="big", name=f"sqps{b}")
                # slice-wise load/cast/square pipeline: the first Gram can
                # start as soon as slice 0 has been cast
                for j in range(NS):
                    nc.sync.dma_start(xt[:, _sl(j)], xT[b][:, _sl(j)])
                    nc.vector.tensor_copy(xb[:, _sl(j)], xt[:, _sl(j)])
                    nc.scalar.activation(
                        xsq[:, _sl(j)], xb[:, _sl(j)], AF.Square
                    )
                    nc.tensor.matmul(
                        sq_ps[:, _sl(j)], lhsT=ones_col_r[:, 0:1], rhs=xsq[:, _sl(j)]
                    )
                sq_row = psm.tile([1, M], F32R, tag="sq_row", name=f"sqrow{b}")
                nc.scalar.copy(sq_row[0:1, :], sq_ps[:, :])
                # sq in chunk layout [128, NT] via DMA + PE transpose so the
                # ACT bias sees the same PE-computed sq as the PSUM d2'
                s16 = psm.tile([NT, 128], F32, tag="s16", name=f"s16_{b}")
                nc.sync.dma_start(s16[:, :], sq_row[0:1, :].bitcast(F32))
                sqc_ps = pp.tile([128, NT], F32, tag="big", name=f"sqcps{b}")
                nc.tensor.transpose(sqc_ps[:, :], s16[:, :], ident[0:NT, 0:NT])
                sqc = psm.tile([128, NT], F32, tag="sqc", name=f"sqc{b}")
                nc.scalar.copy(sqc[:, :], sqc_ps[:, :])
                bias_s = psm.tile([128, NT], F32, tag="bias_s", name=f"biass{b}")
                nc.vector.tensor_scalar_add(bias_s[:, :], sqc[:, :], EPS)
                # msq[p, j] = -sq_j/2 replicated across partitions, consumed
                # by a K=128 (1/128-weighted) PSUM-init matmul: a K=1 rank-1
                # init would drop the PE activity monitor to half utilization
                # and halve the clock for the whole distance pass.
                msqrow = psm.tile([1, M], F32R, tag="rowtmp", bufs=1,
                                  name=f"msqr{b}")
                nc.vector.tensor_scalar_mul(msqrow[0:1, :], sq_row[0:1, :], -0.5)
                msq = pb.tile([128, M], F32R, tag="msq", bufs=2, name=f"msq{b}")
                nc.gpsimd.partition_broadcast(msq[:, :], msqrow[0:1, :])
                s["xb"], s["sq_row"], s["sqc"], s["bias_s"] = (
                    xb, sq_row, sqc, bias_s,
                )
                s["msq"] = msq

            def emit_xn(b):
                s = st[b]
                xn = pb.tile([128, M], F32, tag="xn", bufs=1, name=f"xn{b}")
                nc.sync.dma_start(xn[:, :], xN[b])
                s["xn"] = xn

            # ---------- phase: pass B (sampled dist sums for h) --------------
            # h is a mean over 4.2M off-diag distances; a 2-chunk (256-row)
            # sample estimates it to ~6e-5 relative, far below the spacing
            # of adjacent rho values, so ranks are unaffected.
            def emit_passB(b):
                s = st[b]
                hacc = psm.tile([128, len(HS)], F32, tag="hacc", name=f"hacc{b}")
                for hi, t in enumerate(HS):
                    d2_ps = pp.tile([128, M], F32, tag="big", name=f"d2b{b}_{t}")
                    # Gram first (needs only x), -sq/2 init last: hides the
                    # sq-row/msq latency chain behind the Gram matmuls
                    for j in range(NS):
                        nc.tensor.matmul(
                            d2_ps[:, _sl(j)],
                            lhsT=s["xb"][:, _tb(t)],
                            rhs=s["xb"][:, _sl(j)],
                            start=True,
                            stop=False,
                        )
                    for j in range(NS):
                        nc.tensor.matmul(
                            d2_ps[:, _sl(j)],
                            lhsT=oneon128_r[:, :],
                            rhs=s["msq"][:, _sl(j)],
                            start=False,
                            stop=True,
                        )
                    scr = pb.tile([128, M], F8, tag="scr", bufs=1, name=f"sb{b}_{t}")
                    nc.scalar.activation(
                        scr[:, :],
                        d2_ps[:, :],
                        AF.Sqrt,
                        scale=-2.0,
                        bias=s["bias_s"][:, t : t + 1],
                        accum_out=hacc[:, hi : hi + 1],
                    )
                    if DEBUG_TAPS and t == HS[0]:
                        nc.sync.dma_start(dbg_scr[b], scr[:, :])
                s["hacc"] = hacc
                if DEBUG_TAPS:
                    nc.sync.dma_start(dbg_hacc[b], hacc[:, :])
                    nc.sync.dma_start(dbg_sqrow[b], s["sq_row"][0:1, :].bitcast(F32))
                    nc.sync.dma_start(dbg_biass[b], s["bias_s"][:, :])

            # ---------- phase: h -> f2 = 1/h^2, exp bias -------------------
            def emit_h(b):
                s = st[b]
                hsum = psm.tile([128, 1], F32, tag="hsum", name=f"hsum{b}")
                nc.vector.tensor_reduce(
                    hsum[:, :], s["hacc"][:, :], axis=AX.X, op=ALU.add
                )
                tot_ps = pp.tile([1, 1], F32, tag="big", name=f"tot{b}")
                nc.tensor.matmul(tot_ps[:, :], lhsT=ones_col[:, :], rhs=hsum[:, :])
                ht = psm.tile([1, 1], F32, tag="ht", name=f"ht{b}")
                nc.vector.tensor_scalar(
                    ht[:, :],
                    tot_ps[:, :],
                    1.0 / (len(HS) * 128 * (M - 1)),
                    1e-6,
                    op0=ALU.mult,
                    op1=ALU.max,
                )
                # remove the EPS-induced bias: E[sqrt(d2+EPS)] ~ sqrt(d2)
                # + EPS/(2 h), so h_corr^2 = h^2 - EPS to second order.
                # All-DVE chain (no cross-engine hops until the broadcast).
                h2m = psm.tile([1, 1], F32, tag="h2m", name=f"h2m{b}")
                nc.vector.tensor_mul(h2m[:, :], ht[:, :], ht[:, :])
                h2e = psm.tile([1, 1], F32, tag="h2e", name=f"h2e{b}")
                nc.vector.tensor_scalar_add(h2e[:, :], h2m[:, :], -EPS)
                rh2 = psm.tile([1, 1], F32, tag="rh2", name=f"rh2{b}")
                nc.vector.reciprocal(rh2[:, :], h2e[:, :])
                f_bc = psm.tile([128, 1], F32, tag="fbc", name=f"fbc{b}")
                nc.gpsimd.partition_broadcast(f_bc[:, :], rh2[0:1, :])
                if DEBUG_TAPS:
                    nc.sync.dma_start(dbg_h[b], f_bc[:, :])
                # exp arg = f2*d2' - f2*sq_i/2  (the dropped EPS is a global
                # factor on every rho -> cannot change any comparison)
                bias_e = psm.tile([128, NT], F32, tag="bias_e", name=f"biase{b}")
                nc.vector.tensor_scalar(
                    bias_e[:, :], s["sqc"][:, :], f_bc[:, 0:1], -0.5,
                    op0=ALU.mult, op1=ALU.mult,
                )
                s["f_bc"], s["bias_e"] = f_bc, bias_e

            # ---------- phase: pass C (rho row-sums via exp accumulate) ------
            # PSUM is initialized with the -sq_j/2 row via a K=1 f32r rank-1
            # matmul (exact enough: per-column bias errors would directly
            # perturb rho ranks, so this path stays off bf16), then the
            # bf16 Gram accumulates on top.
            def emit_passC_pre(b):
                s = st[b]
                rho = psm.tile([128, NT], F32, tag="rho", name=f"rho{b}")
                s["rho"] = rho

            def emit_passC_chunk(b, t):
                s = st[b]
                d2_ps = pp.tile([128, M], F32, tag="big", name=f"d2c{b}_{t}")
                for j in range(NS):
                    nc.tensor.matmul(
                        d2_ps[:, _sl(j)], lhsT=s["xb"][:, _tb(t)],
                        rhs=s["xb"][:, _sl(j)], start=True, stop=False,
                    )
                for j in range(NS):
                    nc.tensor.matmul(
                        d2_ps[:, _sl(j)], lhsT=oneon128_r[:, :],
                        rhs=s["msq"][:, _sl(j)], start=False, stop=True,
                    )
                scr = pb.tile([128, M], F8, tag="scr", bufs=1, name=f"sc{b}_{t}")
                nc.scalar.activation(
                    scr[:, :],
                    d2_ps[:, :],
                    AF.Exp,
                    scale=s["f_bc"][:, :],
                    bias=s["bias_e"][:, t : t + 1],
                    accum_out=s["rho"][:, t : t + 1],
                )

            # ---------- per batch building blocks ----------
            def emit_layout(b):
                """rho as an exact [1, M] row and [128, M] broadcast tile."""
                s = st[b]
                rT_ps = pp.tile([NT, 128], F32, tag="big", name=f"rTps{b}")
                nc.tensor.transpose(rT_ps[:, :], s["rho"][:, :], ident[:, :])
                rT = psm.tile([NT, 128], F32, tag="rT", name=f"rT{b}")
                nc.scalar.copy(rT[:, :], rT_ps[:, :])
                rrow = psm.tile([1, M], F32, tag="rowtmp", bufs=1, name=f"rrow{b}")
                nc.sync.dma_start(rrow[0:1, :], rT[:, :])
                rho_bc = pb.tile([128, M], F32, tag="rho_bc", bufs=2, name=f"rbc{b}")
                nc.gpsimd.partition_broadcast(rho_bc[:, :], rrow[0:1, :])
                s["rho_bc"] = rho_bc

            def alloc_lt(b):
                for k in range(NT // 2):
                    lts[b][k] = pb.tile([128, 2, M], F8, tag="lt", bufs=14,
                                        name=f"lt{b}_{k}")

            def emit_ltgen(b, t):
                """one fp8 comparison tile L[p, j] = [rho_j > rho_{t*128+p}],
                written into half t%2 of the chunk-pair tile t//2."""
                s = st[b]
                nc.vector.tensor_scalar(
                    lts[b][t // 2][:, t % 2, :], s["rho_bc"][:, :],
                    s["rho"][:, t : t + 1], None, op0=ALU.is_gt,
                )

            def emit_colsum_mm(b):
                """ranks r_i = #{k: rho_k < rho_i}: column-sum the stored
                compare tiles on the PE (fp8 DoubleRow: one chunk pair per
                pass)."""
                s = st[b]
                r_ps = pp.tile([128, M], F32, tag="big", name=f"rps{b}")
                for k in range(NT // 2):
                    for j in range(NS):
                        nc.tensor.matmul(
                            r_ps[:, _sl(j)], lhsT=ones8[:, :, :],
                            rhs=lts[b][k][:, :, _sl(j)],
                            start=(k == 0), stop=(k == NT // 2 - 1),
                            perf_mode=DR,
                        )
                s["r_ps"] = r_ps

            def emit_colsum_post(b):
                """move the rank row to chunk layout [128, NT]."""
                s = st[b]
                r_sb = psm.tile([1, M], F32, tag="rowtmp", bufs=1, name=f"rsb{b}")
                nc.vector.tensor_copy(r_sb[0:1, :], s["r_ps"][0:1, :])
                r16 = psm.tile([NT, 128], F32, tag="r16", name=f"r16_{b}")
                nc.sync.dma_start(r16[:, :], r_sb[0:1, :])
                rc_ps = pp.tile([128, NT], F32, tag="big", name=f"rcps{b}")
                nc.tensor.transpose(rc_ps[:, :], r16[:, :], ident[0:NT, 0:NT])
                racc = psm.tile([128, NT], F32, tag="racc", name=f"racc{b}")
                nc.scalar.copy(racc[:, :], rc_ps[:, :])
                s["racc"] = racc
                if DEBUG_TAPS:
                    nc.sync.dma_start(dbg_racc[b], racc[:, :])

            def emit_rank_chain(b):
                """Dv/dvs/C vectors from ranks."""
                s = st[b]
                racc = s["racc"]
                # engine-grouped to cut cross-engine dependency hops:
                # DVE batch -> ACT batch -> DVE batch
                Dv = psm.tile([128, NT], F32, tag="Dv", name=f"Dv{b}")
                nc.vector.tensor_scalar_add(Dv[:, :], racc[:, :], 1.0)
                g = psm.tile([128, NT], F32, tag="g", name=f"g{b}")
                nc.vector.tensor_scalar(
                    g[:, :], racc[:, :], -1.0, float(M - 1), op0=ALU.mult,
                    op1=ALU.add,
                )
                gm = psm.tile([128, NT], F32, tag="gm", name=f"gm{b}")
                nc.vector.tensor_scalar_max(gm[:, :], g[:, :], 1.0)
                inv = psm.tile([128, NT], F32, tag="inv", name=f"inv{b}")
                nc.vector.reciprocal(inv[:, :], gm[:, :])
                lnDv = psm.tile([128, NT], F32, tag="lnDv", name=f"lnDv{b}")
                nc.scalar.activation(lnDv[:, :], Dv[:, :], AF.Ln)
                lng = psm.tile([128, NT], F32, tag="lng", name=f"lng{b}")
                nc.scalar.activation(lng[:, :], gm[:, :], AF.Ln)
                dvs = psm.tile([128, NT], F32, tag="dvs", name=f"dvs{b}")
                nc.scalar.activation(dvs[:, :], lnDv[:, :], AF.Exp, scale=-0.5)
                inv2 = psm.tile([128, NT], F32, tag="inv2", name=f"inv2{b}")
                nc.vector.tensor_mul(inv2[:, :], inv[:, :], inv[:, :])
                c1 = psm.tile([128, NT], F32, tag="c1", name=f"c1{b}")
                nc.vector.tensor_scalar(
                    c1[:, :], lng[:, :], -1.0, H_M - GAMMA, op0=ALU.mult, op1=ALU.add
                )
                c2 = psm.tile([128, NT], F32, tag="c2", name=f"c2{b}")
                nc.vector.scalar_tensor_tensor(
                    c2[:, :], in0=inv[:, :], scalar=-0.5, in1=c1[:, :],
                    op0=ALU.mult, op1=ALU.add,
                )
                Cv = psm.tile([128, NT], F32, tag="Cv", name=f"Cv{b}")
                nc.vector.scalar_tensor_tensor(
                    Cv[:, :], in0=inv2[:, :], scalar=1.0 / 12.0, in1=c2[:, :],
                    op0=ALU.mult, op1=ALU.add,
                )
                dvsC = psm.tile([128, NT], F32, tag="dvsC", name=f"dvsC{b}")
                nc.vector.tensor_mul(dvsC[:, :], dvs[:, :], Cv[:, :])
                s["dvs"], s["dvsC"] = dvs, dvsC

            def emit_uvT(b):
                """u, v scaled copies; T column-sums; dvs/dvsC broadcasts."""
                s = st[b]
                dvs, dvsC = s["dvs"], s["dvsC"]
                u = pb.tile([128, M], BF16, tag="u", bufs=3, name=f"u{b}")
                v = pb.tile([128, M], BF16, tag="v", bufs=3, name=f"v{b}")
                for t in range(NT):
                    nc.vector.tensor_scalar(
                        u[:, _tb(t)], s["xn"][:, _tb(t)], dvs[:, t : t + 1], None,
                        op0=ALU.mult,
                    )
                    nc.vector.tensor_scalar(
                        v[:, _tb(t)], s["xn"][:, _tb(t)], dvsC[:, t : t + 1], None,
                        op0=ALU.mult,
                    )
                s["u"], s["v"] = u, v
                T_ps = pp.tile([128, 2], F32, tag="big", name=f"Tps{b}")
                for t in range(NT):
                    nc.tensor.matmul(
                        T_ps[:, :], lhsT=u[:, _tb(t)], rhs=ones2_b[:, :],
                        start=(t == 0), stop=(t == NT - 1),
                    )
                T_sb = psm.tile([128, 1], F32, tag="T_sb", name=f"Tsb{b}")
                nc.scalar.copy(T_sb[:, :], T_ps[:, 0:1])
                s["T_sb"] = T_sb

                stk = psm.tile([128, 2 * NT], F32, tag="stk", name=f"stk{b}")
                nc.vector.tensor_copy(stk[:, 0:NT], dvs[:, :])
                # negated: the projection accumulates W@(dvs*P1) + W@(-dvsC*(P2-T))
                nc.vector.tensor_scalar_mul(stk[:, NT : 2 * NT], dvsC[:, :], -1.0)
                stT_ps = pp.tile([2 * NT, 128], F32, tag="big", name=f"stTps{b}")
                nc.tensor.transpose(stT_ps[:, :], stk[:, :], ident[:, :])
                stT = psm.tile([2 * NT, 128], F32R, tag="stT", name=f"stT{b}")
                nc.vector.tensor_copy(stT[:, :], stT_ps[:, :])
                dvs_row = psm.tile([1, M], F32R, tag="rowtmp", bufs=1,
                                   name=f"dr{b}")
                nc.sync.dma_start(dvs_row[0:1, :], stT[0:NT, :])
                dvsC_row = psm.tile([1, M], F32R, tag="rowtmp", bufs=1,
                                    name=f"cr{b}")
                nc.sync.dma_start(dvsC_row[0:1, :], stT[NT : 2 * NT, :])

                dvs_bc = pb.tile([128, M], F32, tag="dvs_bc", bufs=1, name=f"db{b}")
                nc.gpsimd.partition_broadcast(
                    dvs_bc[:, :], dvs_row[0:1, :].bitcast(F32)
                )
                dvsC_bc = pb.tile([128, M], F32, tag="dvsC_bc", bufs=1,
                                  name=f"cb{b}")
                nc.gpsimd.partition_broadcast(
                    dvsC_bc[:, :], dvsC_row[0:1, :].bitcast(F32)
                )
                s["dvs_bc"], s["dvsC_bc"] = dvs_bc, dvsC_bc

            def emit_l2_half(b, jh, P12_ps, ltgen_tail=None):
                """one j-half of the propagation: P12 packs [P2h | P1h];
                P2h[:, j] = sum_i u_i lt[i,j], P1h likewise with v."""
                s = st[b]
                for t in range(NT):
                    ltp, h = lts[b][t // 2], t % 2
                    for js in range(2):
                        sl = slice(js * 512, (js + 1) * 512)
                        lsl = slice(jh * 1024 + js * 512, jh * 1024 + (js + 1) * 512)
                        nc.tensor.matmul(
                            P12_ps[:, sl], lhsT=s["u"][:, _tb(t)],
                            rhs=ltp[:, h, lsl],
                            start=(t == 0), stop=(t == NT - 1),
                        )
                    for js in range(2):
                        sl = slice(1024 + js * 512, 1024 + (js + 1) * 512)
                        lsl = slice(jh * 1024 + js * 512, jh * 1024 + (js + 1) * 512)
                        nc.tensor.matmul(
                            P12_ps[:, sl], lhsT=s["v"][:, _tb(t)],
                            rhs=ltp[:, h, lsl],
                            start=(t == 0), stop=(t == NT - 1),
                        )
                    if ltgen_tail is not None:
                        ltgen_tail(t)

            def emit_z_half(b, jh, P12_ps):
                """zt1_h = -(dvs*C)_i*(P2h - T_c), zt2_h = dvs_i*P1h."""
                s = st[b]
                hsl = slice(jh * 1024, (jh + 1) * 1024)
                zt2 = pb.tile([128, 1024], F32R, tag="u", bufs=3,
                              name=f"zt2{b}_{jh}")
                nc.vector.scalar_tensor_tensor(
                    zt2[:, :], in0=P12_ps[:, 1024:2048], scalar=0.0,
                    in1=s["dvs_bc"][:, hsl], op0=ALU.bypass, op1=ALU.mult,
                )
                zt1 = pb.tile([128, 1024], F32R, tag="v", bufs=3,
                              name=f"zt1{b}_{jh}")
                nc.vector.scalar_tensor_tensor(
                    zt1[:, :], in0=P12_ps[:, 0:1024], scalar=s["T_sb"][:, 0:1],
                    in1=s["dvsC_bc"][:, hsl], op0=ALU.subtract, op1=ALU.mult,
                )
                s[f"zt{jh}"] = (zt1, zt2)
                if DEBUG_TAPS:
                    zdbg = pb.tile([128, 1024], F32, tag="zdbg", bufs=1,
                                   name=f"zdbg{b}_{jh}")
                    nc.vector.tensor_add(zdbg[:, :], zt2[:, :].bitcast(F32),
                                         zt1[:, :].bitcast(F32))
                    nc.sync.dma_start(dbg_z[b][:, hsl], zdbg[:, :])

            def emit_proj_half(b, jh):
                """yT_h = W @ (zt2_h + zt1_h) via PSUM accumulation; SiLU."""
                s = st[b]
                zt1, zt2 = s[f"zt{jh}"]
                yT_ps = pp.tile([128, 1024], F32, tag="big", name=f"yTps{b}_{jh}")
                for js in range(2):
                    sl = slice(js * 512, (js + 1) * 512)
                    nc.tensor.matmul(
                        yT_ps[:, sl], lhsT=wt_r[:, :], rhs=zt2[:, sl],
                        start=True, stop=False,
                    )
                    nc.tensor.matmul(
                        yT_ps[:, sl], lhsT=wt_r[:, :], rhs=zt1[:, sl],
                        start=False, stop=True,
                    )
                y_sb = pb.tile([128, 1024], F32, tag="y_sb", bufs=2,
                               name=f"ysb{b}_{jh}")
                nc.scalar.activation(y_sb[:, :], yT_ps[:, :], AF.Silu)
                nc.sync.dma_start(
                    yH[b][:, jh * 1024 : (jh + 1) * 1024], y_sb[:, :]
                )

            # ---------- schedule ----------
            # batch-0 mask generation (DVE) rides under batch-1's distance
            # pass (PE/ACT); batch-1 mask generation rides under batch-0's
            # L@u/L@v pass. The propagation/projection runs in j-halves so
            # the z/proj chain of one half overlaps PE work of the next.
            emit_prep(0)
            emit_prep(1)
            # filler matmuls: keep the PE activity monitor boosted while the
            # prep DMA/cast/square chain runs (idle PE halves the clock)
            warm2_ps = pp.tile([128, 512], F32, tag="big", name="warmps2")
            for _w in range(20):
                nc.tensor.matmul(
                    warm2_ps[:, :], lhsT=junk[:, 0:128], rhs=junk[:, :],
                    start=True, stop=True, skip_group_check=True,
                )
            emit_passB(0)
            emit_passB(1)
            emit_h(0)
            emit_xn(0)
            emit_xn(1)
            emit_passC_pre(0)
            for t in range(4):
                emit_passC_chunk(0, t)
            emit_h(1)
            for t in range(4, NT):
                emit_passC_chunk(0, t)
            if DEBUG_TAPS:
                nc.sync.dma_start(dbg_rho[0], st[0]["rho"][:, :])
            emit_layout(0)
            emit_passC_pre(1)
            alloc_lt(0)
            for t in range(NT):
                emit_passC_chunk(1, t)
                emit_ltgen(0, t)
            if DEBUG_TAPS:
                nc.sync.dma_start(dbg_rho[1], st[1]["rho"][:, :])
            emit_layout(1)
            emit_colsum_mm(0)
            emit_colsum_post(0)
            emit_rank_chain(0)
            emit_uvT(0)
            # batch-1 mask tiles: ring slots 8-15 are free (full double
            # buffer), so generation starts as soon as DVE drains the
            # batch-0 scalar chain; colsum(1) then fills the PE between
            # batch-0's two propagation halves, and batch-1's rank chain
            # runs under batch-0's second half.
            alloc_lt(1)
            for t in range(12):
                emit_ltgen(1, t)
            P12a0 = pp.tile([128, M], F32, tag="big", name="P12a0")
            emit_l2_half(0, 0, P12a0)
            emit_z_half(0, 0, P12a0)
            for t in range(12, NT):
                emit_ltgen(1, t)
            emit_colsum_mm(1)
            P12b0 = pp.tile([128, M], F32, tag="big", name="P12b0")
            emit_l2_half(0, 1, P12b0)
            emit_colsum_post(1)
            emit_rank_chain(1)
            emit_z_half(0, 1, P12b0)
            emit_uvT(1)
            P12a1 = pp.tile([128, M], F32, tag="big", name="P12a1")
            emit_l2_half(1, 0, P12a1)
            emit_proj_half(0, 0)
            emit_proj_half(0, 1)
            emit_z_half(1, 0, P12a1)
            P12b1 = pp.tile([128, M], F32, tag="big", name="P12b1")
            emit_l2_half(1, 1, P12b1)
            emit_proj_half(1, 0)
            emit_z_half(1, 1, P12b1)
            emit_proj_half(1, 1)

    nc.compile()
    return nc


_CACHED_NC = None


def _get_nc():
    global _CACHED_NC
    if _CACHED_NC is None:
        _CACHED_NC = build_kernel()
    return _CACHED_NC


def make_in_maps(x, W):
    x = np.asarray(x, dtype=np.float32)
    W = np.asarray(W, dtype=np.float32)
    wt = np.ascontiguousarray(W.T)
    in_maps = []
    for core in range(N_CORES):
        xb = x[core * NB : (core + 1) * NB]                       # [NB, M, C]
        xt = np.ascontiguousarray(xb.transpose(0, 2, 1))          # [NB, C, M]
        # xn[b, p, t*128+c] = x[b, t*128+p, c]
        xn = np.ascontiguousarray(
            xb.reshape(NB, NT, 128, C).transpose(0, 2, 1, 3).reshape(NB, 128, M)
        )
        in_maps.append({"xT": xt, "xN": xn, "WT": wt})
    return in_maps


def unshard_output(results):
    outs = []
    for core in range(N_CORES):
        yh = results[core]["yH"]                                  # [NB, C, M]
        outs.append(yh.transpose(0, 2, 1))                        # [NB, M, C]
    return np.concatenate(outs, axis=0).astype(np.float32)


def run(x, W, trace=False, trace_kwargs=None):
    nc = _get_nc()
    res = run_bass_kernel_spmd(
        nc,
        make_in_maps(x, W),
        list(range(N_CORES)),
        trace=trace,
        **(trace_kwargs or {}),
    )
    return unshard_output(res.results), res


def kernel(x, W):
    y, _ = run(x, W, trace=False)
    return y
